# revision 8
# baseline (speedup 1.0000x reference)
"""ARAP loss kernel for Trainium2 (8 NeuronCores, Bass/Tile).

Strategy (destination-sharded edge-parallel, fixed-slot CSR, no collectives):
  - Host: sort edges by source node i, shard by i-range across 8 cores
    (~400K edges each; core c owns nodes [c*12512, (c+1)*12512)). Lay each
    core's edges into a fixed-slot CSR: node t of partition p owns 64
    consecutive slots in partition row p; pad slots are masked via a
    degree compare on-chip.
  - Device (per core): stream slots in chunks; source-node coords come
    from a dense per-partition node table read with zero-stride broadcast
    APs; j-side coords are streamed per-slot. Compute per-edge w,
    w*(|d|^2+|r|^2) and the 9 entries of w*deform*rest^T, then strided
    tensor_reduce over each node's 64 slots accumulates per-node S
    (3x3 covariance), W and A partials.
  - Rotations: scaled Newton polar iteration on the 3x3 covariances
    (X <- (zeta*X + (zeta*X)^-T)/2 via cofactors); det<0 handled by
    negating the first column (matches U@Vh with the reference sign fix).
  - loss = WEIGHT * (A - 2*B) / W with B = sum_n tr(R_n^T S_n), using
    |R r| = |r| for rotations; per-core partials reduced on host.
"""

import sys

import numpy as np

for _p in ("/opt/trn_rl_repo",):
    if _p not in sys.path:
        sys.path.insert(0, _p)

import concourse.bacc as bacc
import concourse.bass as bass
import concourse.mybir as mybir
import concourse.tile as tile
from concourse.bass_utils import run_bass_kernel_spmd

F32 = mybir.dt.float32
I32 = mybir.dt.int32
OP = mybir.AluOpType
ACT = mybir.ActivationFunctionType
AX = mybir.AxisListType

P = 128
NCORES = 8
N = 100000
SHARD = 12512               # nodes per core (8*12512 >= N)
NPN = 98                    # nodes per partition (128*98 = 12544 >= SHARD)
DPAD = 64                   # slots per node
NCOLS = NPN * DPAD          # 6272 slots per partition row
NCH = 7
C = NCOLS // NCH            # 896 slots per chunk (14 nodes)
NPC = C // DPAD             # nodes per chunk per partition = 7
NEWTON_ITERS = 5
EPS = 1e-8
WEIGHT = 0.01
TINY_DET = 1e-25

_cached = {}


def _build():
    if "nc" in _cached:
        return _cached["nc"]
    nc = bacc.Bacc(None)
    tj = nc.dram_tensor("tj", [NCH, P, C * 6], F32, kind="ExternalInput")
    tn = nc.dram_tensor("tn", [P, NPN * 6], F32, kind="ExternalInput")
    dg = nc.dram_tensor("dg", [P, NPN], F32, kind="ExternalInput")
    io = nc.dram_tensor("io", [P, C], F32, kind="ExternalInput")
    outp = nc.dram_tensor("outp", [P, 4], F32, kind="ExternalOutput")

    with tile.TileContext(nc) as tc:
        with tc.tile_pool(name="sbuf", bufs=2) as pool, \
             tc.tile_pool(name="one", bufs=1) as one:
            tn_t = one.tile([P, NPN * 6], F32, tag="tn_t")
            dg_t = one.tile([P, NPN], F32, tag="dg_t")
            io_t = one.tile([P, C], F32, tag="io_t")
            nc.sync.dma_start(out=tn_t[:], in_=tn[:])
            nc.sync.dma_start(out=dg_t[:], in_=dg[:])
            nc.sync.dma_start(out=io_t[:], in_=io[:])
            S = one.tile([P, NPN * 11], F32, tag="S")

            vfull = one.tile([P, NCOLS], F32, tag="vfull")
            iofull = io_t[:].rearrange("p (t o) -> p t o", o=DPAD) \
                .broadcast(1, NPN).rearrange("p n t o -> p (n t o)") \
                if False else None
            degb_all = dg_t[:].to_broadcast([P, NPN, DPAD])
            # io_t holds one 64-periodic pattern of length C; reuse it per
            # NPC-node group across the full row
            for g in range(NCH):
                nc.vector.tensor_tensor(
                    out=vfull[:, g * C : (g + 1) * C].rearrange(
                        "p (t o) -> p t o", o=DPAD),
                    in0=io_t[:].rearrange("p (t o) -> p t o", o=DPAD),
                    in1=dg_t[:, g * NPC : (g + 1) * NPC].to_broadcast(
                        [P, NPC, DPAD]),
                    op=OP.is_lt)

            for k in range(NCH):
                nb = k * NPC  # node base within partition
                Tj = pool.tile([P, C * 6], F32, tag="Tj")
                nc.sync.dma_start(out=Tj[:], in_=tj[k])

                # zero-stride views broadcasting node data over its 64 slots
                def nview(comp, _nb=nb):
                    v = tn_t[:, (_nb * 6 + comp) : ((_nb + NPC) * 6) : 6]
                    v3 = v.rearrange("p t -> p t 1".replace("1", "()")) \
                        if False else v
                    # build [P, NPC, DPAD] with zero stride on the DPAD dim
                    ap = v.to_broadcast([P, NPC, DPAD])
                    return ap

                degb = dg_t[:, nb : nb + NPC].to_broadcast([P, NPC, DPAD])

                r = [pool.tile([P, C], F32, tag=f"r{a}", name=f"r{a}") for a in range(3)]
                d = [pool.tile([P, C], F32, tag=f"d{a}", name=f"d{a}") for a in range(3)]
                for a in range(3):
                    nc.vector.tensor_tensor(
                        out=r[a][:].rearrange("p (t o) -> p t o", o=DPAD),
                        in0=Tj[:, a::6].rearrange("p (t o) -> p t o", o=DPAD),
                        in1=nview(a), op=OP.subtract)
                    nc.gpsimd.tensor_tensor(
                        out=d[a][:].rearrange("p (t o) -> p t o", o=DPAD),
                        in0=Tj[:, 3 + a :: 6].rearrange("p (t o) -> p t o", o=DPAD),
                        in1=nview(3 + a), op=OP.subtract)
                rn2 = pool.tile([P, C], F32, tag="rn2")
                dn2 = pool.tile([P, C], F32, tag="dn2")
                t0 = pool.tile([P, C], F32, tag="t0")
                t1 = pool.tile([P, C], F32, tag="t1")
                nc.vector.tensor_tensor(out=rn2[:], in0=r[0][:], in1=r[0][:], op=OP.mult)
                nc.vector.tensor_tensor(out=t0[:], in0=r[1][:], in1=r[1][:], op=OP.mult)
                nc.vector.tensor_tensor(out=rn2[:], in0=rn2[:], in1=t0[:], op=OP.add)
                nc.vector.tensor_tensor(out=t0[:], in0=r[2][:], in1=r[2][:], op=OP.mult)
                nc.vector.tensor_tensor(out=rn2[:], in0=rn2[:], in1=t0[:], op=OP.add)
                nc.gpsimd.tensor_tensor(out=dn2[:], in0=d[0][:], in1=d[0][:], op=OP.mult)
                nc.gpsimd.tensor_tensor(out=t1[:], in0=d[1][:], in1=d[1][:], op=OP.mult)
                nc.gpsimd.tensor_tensor(out=dn2[:], in0=dn2[:], in1=t1[:], op=OP.add)
                nc.gpsimd.tensor_tensor(out=t1[:], in0=d[2][:], in1=d[2][:], op=OP.mult)
                nc.gpsimd.tensor_tensor(out=dn2[:], in0=dn2[:], in1=t1[:], op=OP.add)
                # w = v/(sqrt(rn2)+eps)
                w = pool.tile([P, C], F32, tag="w")
                nc.scalar.activation(out=w[:], in_=rn2[:], func=ACT.Sqrt)
                nc.vector.tensor_scalar(
                    out=w[:], in0=w[:], scalar1=EPS, scalar2=None, op0=OP.add)
                nc.vector.reciprocal(out=w[:], in_=w[:])
                nc.vector.tensor_tensor(out=w[:], in0=w[:], in1=vfull[:, k * C : (k + 1) * C], op=OP.mult)

                def red(src_ap, comp, _nb=nb):
                    src3 = src_ap.rearrange("p (t o) -> p t o", o=DPAD)
                    nc.vector.tensor_reduce(
                        out=S[:, (_nb * 11 + comp) : ((_nb + NPC) * 11) : 11],
                        in_=src3, axis=AX.X, op=OP.add)

                red(w[:], 9)
                nc.vector.tensor_tensor(out=t0[:], in0=rn2[:], in1=dn2[:], op=OP.add)
                nc.vector.tensor_tensor(out=t0[:], in0=t0[:], in1=w[:], op=OP.mult)
                red(t0[:], 10)
                for a in range(3):
                    wd = pool.tile([P, C], F32, tag="wd", name="wd")
                    nc.vector.tensor_tensor(out=wd[:], in0=w[:], in1=d[a][:], op=OP.mult)
                    for b in range(3):
                        nc.vector.tensor_tensor(
                            out=t1[:], in0=wd[:], in1=r[b][:], op=OP.mult)
                        red(t1[:], 3 * a + b)

            # ---- rotations + loss partials ----
            NC_ = NPN

            def sview(kk):
                return S[:, kk::11]

            def nt(tag):
                return one.tile([P, NC_], F32, tag=tag, name=tag)

            q = nt("q"); tq = nt("tq")
            nc.vector.tensor_tensor(out=q[:], in0=sview(0), in1=sview(0), op=OP.mult)
            for kk in range(1, 9):
                nc.vector.tensor_tensor(out=tq[:], in0=sview(kk), in1=sview(kk), op=OP.mult)
                nc.vector.tensor_tensor(out=q[:], in0=q[:], in1=tq[:], op=OP.add)
            fn = nt("fn")
            nc.scalar.activation(out=fn[:], in_=q[:], func=ACT.Sqrt)
            nc.vector.tensor_scalar(
                out=fn[:], in0=fn[:], scalar1=1e-30, scalar2=None, op0=OP.max)
            sc = nt("sc")
            nc.vector.reciprocal(out=sc[:], in_=fn[:])

            XA = one.tile([P, NC_ * 9], F32, tag="XA")
            XB = one.tile([P, NC_ * 9], F32, tag="XB")
            CF = one.tile([P, NC_ * 9], F32, tag="CF")
            for kk in range(9):
                nc.vector.tensor_tensor(
                    out=XA[:, kk::9], in0=sview(kk), in1=sc[:], op=OP.mult)

            det = nt("det"); ad = nt("ad"); msk = nt("msk")
            zeta = nt("zeta"); ih = nt("ih"); u0 = nt("u0"); u1 = nt("u1")
            flip = nt("flip")
            cof = []
            for a in range(3):
                a1, a2 = (a + 1) % 3, (a + 2) % 3
                for b in range(3):
                    b1, b2 = (b + 1) % 3, (b + 2) % 3
                    cof.append((3 * a + b, 3 * a1 + b1, 3 * a2 + b2,
                                3 * a1 + b2, 3 * a2 + b1))

            X, Xn = XA, XB
            for it in range(NEWTON_ITERS):
                def xv(kk, _X=X):
                    return _X[:, kk::9]
                for (cidx, p1, p2, m1, m2) in cof:
                    nc.vector.tensor_tensor(out=u0[:], in0=xv(p1), in1=xv(p2), op=OP.mult)
                    nc.vector.tensor_tensor(out=u1[:], in0=xv(m1), in1=xv(m2), op=OP.mult)
                    nc.vector.tensor_tensor(out=CF[:, cidx::9], in0=u0[:], in1=u1[:],
                                            op=OP.subtract)
                nc.vector.tensor_tensor(out=det[:], in0=xv(0), in1=CF[:, 0::9], op=OP.mult)
                nc.vector.tensor_tensor(out=u0[:], in0=xv(1), in1=CF[:, 1::9], op=OP.mult)
                nc.vector.tensor_tensor(out=det[:], in0=det[:], in1=u0[:], op=OP.add)
                nc.vector.tensor_tensor(out=u0[:], in0=xv(2), in1=CF[:, 2::9], op=OP.mult)
                nc.vector.tensor_tensor(out=det[:], in0=det[:], in1=u0[:], op=OP.add)
                if it == 0:
                    nc.vector.tensor_scalar(
                        out=flip[:], in0=det[:], scalar1=0.0, scalar2=None, op0=OP.is_lt)
                nc.scalar.activation(out=ad[:], in_=det[:], func=ACT.Abs)
                nc.vector.tensor_scalar(
                    out=msk[:], in0=ad[:], scalar1=TINY_DET, scalar2=None, op0=OP.is_lt)
                nc.vector.tensor_tensor(out=det[:], in0=det[:], in1=msk[:], op=OP.add)
                nc.vector.tensor_tensor(out=ad[:], in0=ad[:], in1=msk[:], op=OP.add)
                nc.scalar.activation(out=u0[:], in_=ad[:], func=ACT.Ln)
                nc.scalar.activation(out=zeta[:], in_=u0[:], func=ACT.Exp,
                                     scale=-1.0 / 3.0)
                nc.vector.tensor_tensor(out=u0[:], in0=zeta[:], in1=det[:], op=OP.mult)
                nc.vector.reciprocal(out=ih[:], in_=u0[:])
                nc.vector.tensor_scalar(
                    out=ih[:], in0=ih[:], scalar1=0.5, scalar2=None, op0=OP.mult)
                nc.vector.tensor_scalar(
                    out=zeta[:], in0=zeta[:], scalar1=0.5, scalar2=None, op0=OP.mult)
                for kk in range(9):
                    nc.vector.tensor_tensor(out=u0[:], in0=xv(kk), in1=zeta[:], op=OP.mult)
                    nc.vector.tensor_tensor(out=u1[:], in0=CF[:, kk::9], in1=ih[:], op=OP.mult)
                    nc.vector.tensor_tensor(out=Xn[:, kk::9], in0=u0[:], in1=u1[:], op=OP.add)
                X, Xn = Xn, X

            bfull = nt("bfull"); bcol = nt("bcol")
            nc.vector.tensor_tensor(out=bfull[:], in0=X[:, 0::9], in1=sview(0), op=OP.mult)
            for kk in range(1, 9):
                nc.vector.tensor_tensor(out=u0[:], in0=X[:, kk::9], in1=sview(kk), op=OP.mult)
                nc.vector.tensor_tensor(out=bfull[:], in0=bfull[:], in1=u0[:], op=OP.add)
            nc.vector.tensor_tensor(out=bcol[:], in0=X[:, 0::9], in1=sview(0), op=OP.mult)
            for a in (3, 6):
                nc.vector.tensor_tensor(out=u0[:], in0=X[:, a::9], in1=sview(a), op=OP.mult)
                nc.vector.tensor_tensor(out=bcol[:], in0=bcol[:], in1=u0[:], op=OP.add)
            nc.vector.tensor_tensor(out=bcol[:], in0=bcol[:], in1=flip[:], op=OP.mult)
            nc.vector.tensor_scalar(
                out=bcol[:], in0=bcol[:], scalar1=2.0, scalar2=None, op0=OP.mult)
            nc.vector.tensor_tensor(out=bfull[:], in0=bfull[:], in1=bcol[:], op=OP.subtract)

            out_t = one.tile([P, 4], F32, tag="out_t")
            nc.vector.memset(out_t[:], 0.0)
            nc.vector.tensor_reduce(out=out_t[:, 0:1], in_=sview(9), axis=AX.X, op=OP.add)
            nc.vector.tensor_reduce(out=out_t[:, 1:2], in_=sview(10), axis=AX.X, op=OP.add)
            nc.vector.tensor_reduce(out=out_t[:, 2:3], in_=bfull[:], axis=AX.X, op=OP.add)
            nc.sync.dma_start(out=outp[:], in_=out_t[:])

    nc.finalize()
    _cached["nc"] = nc
    return nc


def _prep(mu0, mu, edge_idx):
    i = np.asarray(edge_idx[0], dtype=np.int64)
    j = np.asarray(edge_idx[1], dtype=np.int64)
    T6 = np.concatenate([np.asarray(mu0, np.float32),
                         np.asarray(mu, np.float32)], axis=1)  # [N, 6]
    order = np.argsort(i, kind="stable")
    iso = i[order]
    jso = j[order]
    bounds = np.searchsorted(iso, np.arange(NCORES + 1) * SHARD)
    iota = np.tile(np.tile(np.arange(DPAD, dtype=np.float32), NPC)[None, :],
                   (P, 1))
    in_maps = []
    for c in range(NCORES):
        lo, hi = int(bounds[c]), int(bounds[c + 1])
        loc = iso[lo:hi] - c * SHARD          # sorted, [0, SHARD)
        jj = jso[lo:hi]
        first = np.searchsorted(loc, loc)
        occ = np.arange(hi - lo) - first      # occurrence rank within node
        if occ.size and occ.max() >= DPAD:
            raise ValueError(f"max degree {occ.max()+1} exceeds DPAD={DPAD}")
        pp = loc // NPN
        tt = loc % NPN
        col = tt * DPAD + occ
        tjarr = np.zeros((P, NCOLS, 6), np.float32)
        tjarr[pp, col] = T6[jj]
        tjarr = np.ascontiguousarray(
            tjarr.reshape(P, NCH, C, 6).transpose(1, 0, 2, 3)).reshape(NCH, P, C * 6)
        tnarr = np.zeros((P * NPN, 6), np.float32)
        gids = c * SHARD + np.arange(P * NPN)
        valid = gids < min((c + 1) * SHARD, N)
        tnarr[valid] = T6[gids[valid]]
        deg = np.zeros(P * NPN, np.float32)
        cnt = np.bincount(loc, minlength=SHARD).astype(np.float32)
        deg[:SHARD] = cnt[:SHARD]
        in_maps.append(dict(
            tj=tjarr,
            tn=tnarr.reshape(P, NPN * 6),
            dg=deg.reshape(P, NPN),
            io=iota,
        ))
    return in_maps


def kernel(mu0, mu, edge_idx, _trace=False):
    nc = _build()
    in_maps = _prep(np.asarray(mu0), np.asarray(mu), np.asarray(edge_idx))
    res = run_bass_kernel_spmd(nc, in_maps, core_ids=list(range(NCORES)),
                               trace=_trace)
    Wt = At = Bt = 0.0
    for cc in range(NCORES):
        o = res.results[cc]["outp"].astype(np.float64)
        Wt += o[:, 0].sum()
        At += o[:, 1].sum()
        Bt += o[:, 2].sum()
    loss = WEIGHT * (At - 2.0 * Bt) / Wt
    if _trace:
        kernel.last_exec_time_ns = res.exec_time_ns
        kernel.last_results = res
    return np.float32(loss)
